# revision 20
# baseline (speedup 1.0000x reference)
"""Bass/Trainium2 kernel for nn_Bilinear (out[b,n,i] = enc[b,n,i,:] @ W @ hidden[b,:] + bias).

Sharding: data-parallel over B. 8 cores, one batch element each.
Per core:
  stage 1 (TensorE): v[j] = sum_k W[j,k] * h[k].  Host feeds Wt = W.T so the
    contraction dim k sits on SBUF partitions; Wt streams in as 8 chunked
    0.5 MiB DMAs (h/bias first, so matmuls only wait on their Wt chunk) and
    16 small matmuls pipeline behind them, accumulating v into PSUM.  v is
    partition-broadcast on the PE (ones[1,128].T @ v[1,512] -> [128,512])
    so no DMA sits on the v critical path.
  stage 2 (VectorE + ScalarE): stream enc rows as [128, 4, 1024] tiles
    (2 MiB DMAs); per 128-row block, 1-in-4 blocks use the fused custom-DVE
    TENSOR_TENSOR_REDUCE and the rest use DVE-mul + ScalarE accumulate-Copy,
    balancing both engines below the DMA rate so the kernel stays
    DMA-paced end to end.  The last chunks are tapered (1 MiB) to shorten
    the compute trail after the stream ends.  Bias is added once at the end.
Output is written per-core as out[b].T ([128 i, 64 n]); host transposes back.
"""

import numpy as np

B, N, I, H = 8, 64, 128, 1024
P = 128
NI = N * I  # 8192 rows per core
N_CORES = 8

_NC_CACHE = {}
LAST_RESULTS = None


def _build(ni_rows=NI, ebufs=8):
    import concourse.bacc as bacc
    import concourse.mybir as mybir
    import concourse.tile as tile
    from concourse import dve_ops

    f32 = mybir.dt.float32
    KB = H // P  # k blocks for stage 1
    n_blocks = ni_rows // P
    # chunk schedule in 128-row blocks: 2 MiB (4 blocks) for the bulk,
    # 1 MiB (2 blocks) for the last few to shorten the trailing compute
    tail_blocks = 8 if n_blocks > 8 else 0
    bulk = n_blocks - tail_blocks
    chunks = [4] * (bulk // 4) + [2] * (tail_blocks // 2)
    assert sum(chunks) == n_blocks

    nc = bacc.Bacc(
        "TRN2",
        target_bir_lowering=False,
        debug=False,
        num_devices=N_CORES,
    )
    enc = nc.declare_dram_parameter("enc", [ni_rows, H], f32, isOutput=False)
    hh = nc.declare_dram_parameter("h", [P, KB], f32, isOutput=False)
    wt = nc.declare_dram_parameter("wt", [H, H], f32, isOutput=False)
    bb = nc.declare_dram_parameter("bias", [1, 1], f32, isOutput=False)
    out = nc.declare_dram_parameter("out_t", [P, n_blocks], f32, isOutput=True)

    with tile.TileContext(nc) as tc:
        with (
            tc.tile_pool(name="const", bufs=1) as const,
            tc.tile_pool(name="epool", bufs=ebufs) as epool,
            tc.tile_pool(name="ppool", bufs=3) as ppool,
            tc.tile_pool(name="vpsum", bufs=1, space="PSUM") as vpsum,
        ):
            # ---- stage 1: v[j] = sum_k Wt[k,j] h[k] ----
            h_col = const.tile([P, KB], f32)
            nc.sync.dma_start(out=h_col[:], in_=hh[:, :])
            bias_col = const.tile([P, 1], f32)
            nc.sync.dma_start(out=bias_col[:], in_=bb[:, :].to_broadcast((P, 1)))
            wt_sbs = []
            for kb in range(KB):
                wt_kb = const.tile([P, H], f32, name=f"wt{kb}", tag=f"wt{kb}")
                nc.sync.dma_start(out=wt_kb[:], in_=wt[kb * P : (kb + 1) * P, :])
                wt_sbs.append(wt_kb)
            ones = const.tile([1, P], f32)
            nc.vector.memset(ones[:], 1.0)

            v_flat = const.tile([1, H], f32)
            vps = [
                vpsum.tile([1, 512], f32, name=f"vp{jc}", tag=f"vp{jc}")
                for jc in range(H // 512)
            ]
            for kb in range(KB):
                for jc in range(H // 512):
                    nc.tensor.matmul(
                        vps[jc][:],
                        h_col[:, kb : kb + 1],
                        wt_sbs[kb][:, jc * 512 : (jc + 1) * 512],
                        start=(kb == 0),
                        stop=(kb == KB - 1),
                    )
            for jc in range(H // 512):
                nc.scalar.activation(
                    v_flat[:, jc * 512 : (jc + 1) * 512],
                    vps[jc][:],
                    mybir.ActivationFunctionType.Copy,
                )
            # partition-broadcast v on the PE: ones[1,P].T @ v[1,512] -> [P,512]
            v_rep = const.tile([P, H], f32)
            for jc in range(H // 512):
                bc = vpsum.tile([P, 512], f32, name=f"bc{jc}", tag=f"bc{jc}")
                nc.tensor.matmul(
                    bc[:],
                    ones[:],
                    v_flat[:, jc * 512 : (jc + 1) * 512],
                    start=True,
                    stop=True,
                )
                nc.scalar.activation(
                    v_rep[:, jc * 512 : (jc + 1) * 512],
                    bc[:],
                    mybir.ActivationFunctionType.Copy,
                )

            # ---- stage 2: out[col*128+p] = sum_j enc[row, j] * v[j] ----
            # Per 4 blocks, 1 uses the fused all-DVE TTR and 3 use DVE-mul +
            # ScalarE accumulate-Copy, balancing the two engines (~5 us per
            # 2 MiB chunk each) under the ~5.5 us/chunk DMA.
            out_sb = const.tile([P, n_blocks], f32)
            dummy = const.tile([P, 1], f32)
            enc_b = enc[:, :].rearrange("(blk p) j -> blk p j", p=P)
            col = 0
            for ci, C in enumerate(chunks):
                e_tile = epool.tile([P, 4, H], f32, name=f"e{ci}", tag="e")
                nc.sync.dma_start(
                    out=e_tile[:, :C],
                    in_=enc_b[col : col + C].rearrange("blk p j -> p blk j"),
                )
                for c in range(C):
                    if col % 2 == 0:
                        nc.vector._custom_dve(
                            dve_ops.TENSOR_TENSOR_REDUCE,
                            out=dummy[:].broadcast_to((P, H)),
                            in0=e_tile[:, c],
                            in1=v_rep[:],
                            s0=0.0,
                            s1=1.0,
                            accum_out=out_sb[:, col : col + 1],
                        )
                    else:
                        prod = ppool.tile([P, H], f32)
                        nc.vector.tensor_mul(prod[:], e_tile[:, c], v_rep[:])
                        nc.scalar.activation(
                            prod[:],
                            prod[:],
                            mybir.ActivationFunctionType.Copy,
                            accum_out=out_sb[:, col : col + 1],
                        )
                    col += 1
            # bias once over the whole [128, n_blocks] result
            nc.vector.tensor_scalar_add(out_sb[:], out_sb[:], bias_col[:])
            nc.sync.dma_start(out=out[:, :], in_=out_sb[:])
    nc.compile()
    return nc


def _get_nc():
    if "nc" not in _NC_CACHE:
        _NC_CACHE["nc"] = _build()
    return _NC_CACHE["nc"]


def kernel(hidden=None, encoder_hiddens=None, input_lengths=None, W=None, b=None):
    global LAST_RESULTS
    from concourse.bass_utils import run_bass_kernel_spmd

    hidden = np.asarray(hidden, dtype=np.float32)
    enc = np.asarray(encoder_hiddens, dtype=np.float32)
    W_ = np.asarray(W, dtype=np.float32)
    b_ = np.asarray(b, dtype=np.float32).reshape(1, 1)
    wt = np.ascontiguousarray(W_.T)

    nc = _get_nc()
    KB = H // P
    in_maps = []
    for core in range(N_CORES):
        in_maps.append(
            {
                "enc": np.ascontiguousarray(enc[core].reshape(NI, H)),
                "h": np.ascontiguousarray(hidden[core].reshape(KB, P).T),
                "wt": wt,
                "bias": b_,
            }
        )
    res = run_bass_kernel_spmd(nc, in_maps, core_ids=list(range(N_CORES)))
    LAST_RESULTS = res
    out = np.stack([res.results[i]["out_t"].T for i in range(N_CORES)])
    return np.ascontiguousarray(out.astype(np.float32))
